# revision 3
# baseline (speedup 1.0000x reference)
"""Trainium2 Bass kernel for CIN layer:
    out[b,c,d] = sum_{h,m} W[c, h*M+m] * xk[b,h,d] * x0[b,m,d] + bias[c]

Shapes (hardcoded): x0 [512,40,64] f32, xk [512,128,64] f32,
W [128,5120] f32, b [128] f32 -> out [512,128,64] f32.

Strategy: data-parallel over batch B across 8 cores (64 batches/core).
Per core, columns are the 64*64=4096 (b,d) pairs. The 5120-long
contraction is ordered m-major: chunk m holds rows (m, h=0..127).
Per m:
  outer_m[h, col] = xk[h, col] * x0bc_m[col]    (DVE tensor_tensor, bf16)
  psum[g] += W_mT[h,c].T @ outer_m[:, g*512:...] (PE, accumulate over m)
x0bc_m is x0's row m replicated across the 128 partitions; the
replication is produced host-side (pure layout, no arithmetic) and
streamed from HBM by DMA. W is host-pre-transposed to [h, m, c].
Bias-add is fused into the PSUM->SBUF eviction on ScalarE.
"""

import numpy as np
import ml_dtypes

B, M, H, D, C = 512, 40, 128, 64, 128
N_CORES = 8
BC = B // N_CORES          # 64 batches per core
COLS = BC * D              # 4096 (b,d) columns per core
NG = 8                     # PSUM groups
GW = COLS // NG            # 512 columns per group

_cache = {}


def _build():
    import concourse.bacc as bacc
    import concourse.mybir as mybir
    from concourse.tile import TileContext

    f32 = mybir.dt.float32
    bf16 = mybir.dt.bfloat16

    nc = bacc.Bacc("TRN2", debug=False, num_devices=N_CORES)

    xk_d = nc.dram_tensor("xk_in", [BC, H, D], f32, kind="ExternalInput")
    x0r_d = nc.dram_tensor("x0rep_in", [M, 128, COLS], bf16, kind="ExternalInput")
    wT_d = nc.dram_tensor("wT_in", [H, M, C], f32, kind="ExternalInput")
    bias_d = nc.dram_tensor("bias_in", [C, 1], f32, kind="ExternalInput")
    out_d = nc.dram_tensor("out", [BC, C, D], f32, kind="ExternalOutput")

    with TileContext(nc) as tc:
        with (
            tc.tile_pool(name="const", bufs=1) as cpool,
            tc.tile_pool(name="work", bufs=3) as wpool,
            tc.tile_pool(name="outp", bufs=2) as opool,
            tc.tile_pool(name="psum", bufs=1, space="PSUM") as ppool,
        ):
            # ---- load + cast constants ----
            xk_f32 = cpool.tile([128, COLS], f32)
            nc.sync.dma_start(out=xk_f32, in_=xk_d.ap().rearrange("b h d -> h b d"))
            xk_sb = cpool.tile([128, COLS], bf16)
            nc.vector.tensor_copy(xk_sb, xk_f32)

            wT_f32 = cpool.tile([128, M * C], f32)
            nc.sync.dma_start(out=wT_f32, in_=wT_d.ap().rearrange("h m c -> h (m c)"))
            w_sb = cpool.tile([128, M * C], bf16)
            nc.vector.tensor_copy(w_sb, wT_f32)

            bias_sb = cpool.tile([128, 1], f32)
            nc.sync.dma_start(out=bias_sb, in_=bias_d.ap())

            psums = []
            for g in range(NG):
                ps = ppool.tile([128, GW], f32, name=f"ps{g}", tag=f"ps{g}")
                psums.append(ps)

            # ---- main loop over the 40 m-chunks ----
            for m in range(M):
                x0bc = wpool.tile([128, COLS], bf16, name=f"x0bc{m}", tag="x0bc")
                nc.sync.dma_start(out=x0bc, in_=x0r_d.ap()[m])
                outer = wpool.tile([128, COLS], bf16, name=f"outer{m}", tag="outer")
                nc.vector.tensor_mul(outer, xk_sb, x0bc)
                for g in range(NG):
                    nc.tensor.matmul(
                        psums[g],
                        lhsT=w_sb[:, m * C:(m + 1) * C],
                        rhs=outer[:, g * GW:(g + 1) * GW],
                        start=(m == 0),
                        stop=(m == M - 1),
                    )

            # ---- bias add + store ----
            out_ap = out_d.ap().rearrange("b c d -> c b d")
            bpg = NG and BC // NG  # batches per group
            for g in range(NG):
                out_sb = opool.tile([128, GW], f32, name=f"osb{g}", tag="osb")
                nc.scalar.activation(
                    out_sb,
                    psums[g],
                    mybir.ActivationFunctionType.Identity,
                    bias=bias_sb[:, 0:1],
                    scale=1.0,
                )
                nc.sync.dma_start(
                    out=out_ap[:, g * bpg:(g + 1) * bpg, :], in_=out_sb
                )

    nc.compile()
    return nc


def _prep_host(x0, xk, W, b):
    """Host-side layout prep (no arithmetic): shard, transpose, replicate."""
    wT = np.ascontiguousarray(
        W.reshape(C, H, M).transpose(1, 2, 0)
    )  # [h, m, c] f32
    bias = np.ascontiguousarray(b.reshape(C, 1)).astype(np.float32)
    in_maps = []
    for k in range(N_CORES):
        x0s = x0[k * BC:(k + 1) * BC]            # [BC, M, D]
        xks = np.ascontiguousarray(xk[k * BC:(k + 1) * BC])  # [BC, H, D]
        x0rows = np.ascontiguousarray(x0s.transpose(1, 0, 2)).reshape(M, COLS)
        x0rep = np.ascontiguousarray(
            np.broadcast_to(
                x0rows.astype(ml_dtypes.bfloat16)[:, None, :], (M, 128, COLS)
            )
        )
        in_maps.append(
            {
                "xk_in": xks.astype(np.float32, copy=False),
                "x0rep_in": x0rep,
                "wT_in": wT,
                "bias_in": bias,
            }
        )
    return in_maps


def _run(in_maps, **kwargs):
    from concourse import bass_utils

    if "nc" not in _cache:
        _cache["nc"] = _build()
    return bass_utils.run_bass_kernel_spmd(
        _cache["nc"], in_maps, core_ids=list(range(N_CORES)), **kwargs
    )


def kernel(x0, xk, W, b, _bench=[None]):
    x0 = np.asarray(x0, dtype=np.float32)
    xk = np.asarray(xk, dtype=np.float32)
    W = np.asarray(W, dtype=np.float32)
    b = np.asarray(b, dtype=np.float32)
    in_maps = _prep_host(x0, xk, W, b)
    res = _run(in_maps)
    _bench[0] = res
    out = np.concatenate([r["out"] for r in res.results], axis=0)
    return out.astype(np.float32, copy=False)
